# revision 1
# baseline (speedup 1.0000x reference)
"""DecoderAttention Bass/Tile kernel for TRN2, batch-parallel over 8 NeuronCores.

Each core handles one batch element:
  q = enc @ Qs + Qbs ; k = enc @ Ks + Kbs ; v = nrp @ Vs + Vbs   (per head)
  scores = q k^T / sqrt(64), causal mask (-1e5), softmax
  out = (attn @ v) @ O + Ob

Layout strategy (matmuls in fp32r at full PE rate):
  - enc/nrp transposed on-device (PE transpose) to [d, s]
  - weights pre-packed host-side to [d, (h dh)]; Vs padded to [d, 16*65]
    with a ones column per head so attn@v also produces softmax row sums
  - scoresT [m, q] per head so exp output feeds attn@v without transposing
  - causal diagonal blocks masked by a GpSimd affine_select zeroing exp output
  - exp folds the 1/sqrt(d_head) scale; no max subtraction (scores are O(1),
    masked entries become exactly 0)
  - q/k projections for pair g+1 are interleaved into pair g's attention as
    PE filler work, so the tensor engine never idles long enough for the HAM
    clock gate to re-throttle it to 1.2 GHz
  - softmax division deferred: one batched reciprocal at the end, broadcast
    across partitions with one-hot selector matmuls
"""

import numpy as np

import concourse.bass as bass
import concourse.mybir as mybir
import concourse.tile as tile
from concourse import bacc
from concourse.bass_utils import run_bass_kernel_spmd
from concourse.masks import make_identity

N_HEADS, D_MODEL, D_HEAD = 16, 1024, 64
BATCH, SEQ = 8, 1024
P = 128
DCH = D_MODEL // P       # 8 contraction chunks
ST = SEQ // P            # 8 seq tiles
PAIRS = N_HEADS // 2     # 8 head pairs
VW = 65                  # v width per head incl. ones column
VTOT = N_HEADS * VW      # 1040
IGNORE = -100000.0
SCALE = 1.0 / np.sqrt(np.float32(D_HEAD))

F32 = mybir.dt.float32
F32R = mybir.dt.float32r
BF16 = mybir.dt.bfloat16
AF = mybir.ActivationFunctionType

_CACHE = {}


def _bank_splits(q0):
    # PSUM-bank-aligned (n0, nw) column splits covering [q0, SEQ)
    if q0 < 512:
        return [(q0, 512 - q0), (512, 512)]
    return [(q0, SEQ - q0)]


def _bcast_row_ap(src, n):
    # DMA access pattern replicating a [n]-element DRAM row to 128 partitions
    return bass.AP(tensor=src.tensor, offset=src.offset, ap=[[0, P], [1, n]])


def _build_program(debug=False):
    nc = bacc.Bacc("TRN2", target_bir_lowering=False, debug=False, num_devices=8)

    enc = nc.dram_tensor("enc", [SEQ, D_MODEL], F32R, kind="ExternalInput").ap()
    nrp = nc.dram_tensor("nrp", [SEQ, D_MODEL], F32R, kind="ExternalInput").ap()
    qst = nc.dram_tensor("qst", [D_MODEL, D_MODEL], F32R, kind="ExternalInput").ap()
    kst = nc.dram_tensor("kst", [D_MODEL, D_MODEL], F32R, kind="ExternalInput").ap()
    vst = nc.dram_tensor("vst", [D_MODEL, VTOT], F32R, kind="ExternalInput").ap()
    ow = nc.dram_tensor("ow", [D_MODEL, D_MODEL], F32R, kind="ExternalInput").ap()
    qb = nc.dram_tensor("qb", [D_MODEL], F32, kind="ExternalInput").ap()
    kb = nc.dram_tensor("kb", [D_MODEL], F32, kind="ExternalInput").ap()
    vb = nc.dram_tensor("vb", [VTOT], F32, kind="ExternalInput").ap()
    ob = nc.dram_tensor("ob", [D_MODEL], F32, kind="ExternalInput").ap()
    out = nc.dram_tensor("out", [SEQ, D_MODEL], F32, kind="ExternalOutput").ap()
    sums_dram = nc.dram_tensor("sums_scratch", [N_HEADS, SEQ], F32).ap()
    rcp_dram = nc.dram_tensor("rcp_scratch", [P, P], F32R).ap()
    dbg = None
    if debug:
        dbg = {
            "qt0": nc.dram_tensor("d_qt0", [P, SEQ], F32, kind="ExternalOutput").ap(),
            "kt0": nc.dram_tensor("d_kt0", [P, SEQ], F32, kind="ExternalOutput").ap(),
            "va0": nc.dram_tensor("d_va0", [P, VTOT], F32, kind="ExternalOutput").ap(),
            "zt": nc.dram_tensor("d_zt", [DCH, P, SEQ], F32, kind="ExternalOutput").ap(),
        }

    with tile.TileContext(nc) as tc:
        _kernel(tc, out, enc, nrp, qst, kst, vst, ow, qb, kb, vb, ob,
                sums_dram=sums_dram, rcp_dram=rcp_dram, dbg=dbg)
    nc.compile()
    return nc


def _kernel(tc, out, enc, nrp, qst, kst, vst, ow, qb, kb, vb, ob,
            sums_dram=None, rcp_dram=None, dbg=None):
    nc = tc.nc

    smalls = tc.alloc_tile_pool(name="smalls", bufs=1)
    identf = smalls.tile([P, P], F32, tag="identf", name="identf")
    make_identity(nc, identf)
    ident = smalls.tile([P, P], F32R, tag="ident", name="ident")
    nc.vector.tensor_copy(ident, identf)
    ident_bf = smalls.tile([P, P], BF16, tag="ident_bf", name="ident_bf")
    make_identity(nc, ident_bf)
    # M0[m, q] = IGNORE where m > q else 0 (strict causal mask, diag block)
    mask_bf = smalls.tile([P, P], BF16, tag="mask_bf", name="mask_bf")
    nc.gpsimd.memset(mask_bf, 0.0)
    nc.gpsimd.affine_select(
        out=mask_bf, in_=mask_bf,
        compare_op=mybir.AluOpType.is_ge,
        fill=IGNORE, base=0,
        pattern=[[1, P]], channel_multiplier=-1,
    )
    vb_bc = smalls.tile([P, VTOT], F32, tag="vb_bc", name="vb_bc")
    nc.sync.dma_start(out=vb_bc, in_=_bcast_row_ap(vb, VTOT))
    ob_bc = smalls.tile([P, D_MODEL], F32, tag="ob_bc", name="ob_bc")
    nc.sync.dma_start(out=ob_bc, in_=_bcast_row_ap(ob, D_MODEL))
    qb_col = smalls.tile([P, PAIRS], F32, tag="qb_col", name="qb_col")
    nc.sync.dma_start(out=qb_col, in_=qb.rearrange("(g p) -> p g", p=P))
    kb_col = smalls.tile([P, PAIRS], F32, tag="kb_col", name="kb_col")
    nc.sync.dma_start(out=kb_col, in_=kb.rearrange("(g p) -> p g", p=P))

    enc_t_pool = tc.alloc_tile_pool(name="encT", bufs=1, side="right")
    nrp_t_pool = tc.alloc_tile_pool(name="nrpT", bufs=1, side="right")
    encT = [enc_t_pool.tile([P, SEQ], F32R, tag=f"encT{c}", name=f"encT{c}") for c in range(DCH)]
    nrpT = [nrp_t_pool.tile([P, SEQ], F32R, tag=f"nrpT{c}", name=f"nrpT{c}") for c in range(DCH)]

    # ---- phase 1: transpose enc and nrp into [d, s] ----
    with tc.tile_pool(name="trin", bufs=2) as trin, \
         tc.tile_pool(name="trps", bufs=1, space="PSUM") as trps:
        for src, dst in ((enc, encT), (nrp, nrpT)):
            for tq in range(0, ST, 4):
                ptiles = [trps.tile([P, 4 * P], F32R, tag=f"tr{c}", name=f"tr{c}") for c in range(DCH)]
                for t in range(tq, tq + 4):
                    s_in = trin.tile([P, D_MODEL], F32R, tag="s_in", name="s_in")
                    nc.sync.dma_start(out=s_in, in_=src[t * P:(t + 1) * P, :])
                    for c in range(DCH):
                        nc.tensor.transpose(
                            ptiles[c][:, (t - tq) * P:(t - tq + 1) * P],
                            s_in[:, c * P:(c + 1) * P],
                            ident,
                        )
                for c in range(DCH):
                    nc.any.tensor_copy(dst[c][:, tq * P:(tq + 4) * P], ptiles[c])

    # ---- phase 2ab: q/k projections (dedicated phase, weights prefetched) ----
    qt_pool = tc.alloc_tile_pool(name="qt", bufs=1)
    kt_pool = tc.alloc_tile_pool(name="kt", bufs=1)
    qt = [qt_pool.tile([P, SEQ], F32R, tag=f"qt{g}", name=f"qt{g}") for g in range(PAIRS)]
    kt = [kt_pool.tile([P, SEQ], F32R, tag=f"kt{g}", name=f"kt{g}") for g in range(PAIRS)]
    with tc.tile_pool(name="wsb", bufs=1) as wsb, \
         tc.tile_pool(name="pproj", bufs=1, space="PSUM") as pproj:
        wqk = {}
        for pfx, wsrc in (("q", qst), ("k", kst)):
            w = [wsb.tile([P, D_MODEL], F32R, tag=f"{pfx}w{c}", name=f"{pfx}w{c}") for c in range(DCH)]
            for c in range(DCH):
                nc.scalar.dma_start(out=w[c], in_=wsrc[c * P:(c + 1) * P, :])
            wqk[pfx] = w
        for pfx, bcol, dsts in (("q", qb_col, qt), ("k", kb_col, kt)):
            w = wqk[pfx]
            for n0 in range(0, SEQ, 512):
                ptiles = [pproj.tile([P, 512], F32, tag=f"pp{g}", name=f"pp{g}") for g in range(PAIRS)]
                for c in range(DCH):
                    for g in range(PAIRS):
                        nc.tensor.matmul(
                            ptiles[g],
                            w[c][:, g * P:(g + 1) * P],
                            encT[c][:, n0:n0 + 512],
                            start=(c == 0), stop=(c == DCH - 1),
                        )
                for g in range(PAIRS):
                    nc.vector.tensor_scalar_add(
                        out=dsts[g][:, n0:n0 + 512],
                        in0=ptiles[g],
                        scalar1=bcol[:, g:g + 1],
                    )

    # ---- phase 2: v projection -> va [m, 16*65] with ones columns ----
    va_pool = tc.alloc_tile_pool(name="va", bufs=1)
    va = [va_pool.tile([P, VTOT], F32R, tag=f"va{t}", name=f"va{t}") for t in range(ST)]
    with tc.tile_pool(name="vsb", bufs=1) as vsb, \
         tc.tile_pool(name="pv", bufs=2, space="PSUM") as pv:
        vw = [vsb.tile([P, VTOT], F32R, tag=f"vw{c}", name=f"vw{c}") for c in range(DCH)]
        for c in range(DCH):
            nc.scalar.dma_start(out=vw[c], in_=vst[c * P:(c + 1) * P, :])
        for t in range(ST):
            pt = pv.tile([P, VTOT], F32, tag="pv", name="pvt")
            for c in range(DCH):
                for n0 in range(0, VTOT, 512):
                    nw = min(512, VTOT - n0)
                    nc.tensor.matmul(
                        pt[:, n0:n0 + nw],
                        nrpT[c][:, t * P:(t + 1) * P],
                        vw[c][:, n0:n0 + nw],
                        start=(c == 0), stop=(c == DCH - 1),
                    )
            # vb_bc has the per-(h,dh) bias, with 1.0 in each ones-column slot;
            # matmul wrote 0 there (vst ones-columns are zero), so add gives 1.0
            nc.vector.tensor_add(va[t], pt, vb_bc)

    nrp_t_pool.release()
    enc_t_pool.release()

    # ---- phase 3: attention, with next pair's q/k projection interleaved ----
    zt_pool = tc.alloc_tile_pool(name="zt", bufs=1)
    zt = [zt_pool.tile([P, SEQ], F32R, tag=f"zt{k}", name=f"zt{k}") for k in range(DCH)]
    osb = tc.alloc_tile_pool(name="osb", bufs=1)
    owt = [osb.tile([P, D_MODEL], F32R, tag=f"ow{k}", name=f"owt{k}") for k in range(DCH)]
    for k in range(DCH):
        nc.scalar.dma_start(out=owt[k], in_=ow[k * P:(k + 1) * P, :])

    with tc.tile_pool(name="attn", bufs=3) as apool, \
         tc.tile_pool(name="rcp", bufs=1) as rpool, \
         tc.tile_pool(name="selp", bufs=1) as selp, \
         tc.tile_pool(name="ps_s", bufs=2, space="PSUM") as spool, \
         tc.tile_pool(name="ps_z", bufs=2, space="PSUM") as zpool:
        # sel[g][j, p] = 1 where j == 2g + p // 64  (K=16 one-hot broadcast)
        sel = []
        for b in range(PAIRS):
            self_f = selp.tile([N_HEADS, P], F32, tag="self", name="self", bufs=2)
            nc.gpsimd.memset(self_f, 0.0)
            nc.gpsimd.affine_select(
                out=self_f.rearrange("j (a c) -> j a c", a=2),
                in_=self_f.rearrange("j (a c) -> j a c", a=2),
                compare_op=mybir.AluOpType.not_equal,
                fill=1.0, base=-2 * b,
                pattern=[[-1, 2], [0, D_HEAD]], channel_multiplier=1,
            )
            s_r = selp.tile([N_HEADS, P], F32R, tag=f"sel{b}", name=f"sel{b}")
            nc.vector.tensor_copy(s_r, self_f)
            sel.append(s_r)

        for h in range(N_HEADS):
            g, off = h // 2, (h % 2) * D_HEAD
            pz = zpool.tile([VW, SEQ], F32, tag="pz", name="pz")

            def av_mms(i, ae):
                q0 = i * P
                for n0, nw in _bank_splits(q0):
                    nc.tensor.matmul(
                        pz[:, n0:n0 + nw],
                        va[i][:, h * VW:(h + 1) * VW],
                        ae[:, n0:n0 + nw],
                        start=(i == 0), stop=(i == ST - 1),
                        skip_group_check=True,
                    )

            pend = None
            for i in range(ST):
                q0 = i * P
                ps = spool.tile([P, SEQ], F32, tag="ps", name="ps")
                ae = apool.tile([P, SEQ], F32R, tag="ae", name="ae")
                for n0, nw in _bank_splits(q0):
                    nc.tensor.matmul(
                        ps[:, n0:n0 + nw],
                        kt[g][off:off + D_HEAD, q0:q0 + P],
                        qt[g][off:off + D_HEAD, n0:n0 + nw],
                        start=True, stop=(n0 != q0),
                        skip_group_check=True,
                    )
                # causal diag mask: accumulate I.T @ M0
                nc.tensor.matmul(
                    ps[:, q0:q0 + P],
                    ident_bf, mask_bf,
                    start=False, stop=True,
                    skip_group_check=True,
                )
                nc.scalar.activation(
                    out=ae[:, q0:SEQ], in_=ps[:, q0:SEQ],
                    func=AF.Exp, scale=float(SCALE),
                )
                # attn@v delayed one chunk so exp latency hides behind PE work
                if pend is not None:
                    av_mms(*pend)
                pend = (i, ae)
            av_mms(*pend)
            # stash unnormalized zT and the denominator row; frees PSUM slots
            nc.vector.tensor_copy(zt[g][off:off + D_HEAD, :], pz[0:D_HEAD, :])
            srow = rpool.tile([1, SEQ], F32, tag="srow", name="srow", bufs=2)
            nc.scalar.copy(out=srow, in_=pz[D_HEAD:VW, :])
            nc.sync.dma_start(out=sums_dram[h:h + 1, :], in_=srow)

        # normalize: reciprocal over the sums reshaped to [128, 128] so the
        # FD-bound iterative divide runs across partitions (1.3us vs 6.5us),
        # then reload in [16, SEQ] layout for the broadcast matmuls
        s128 = rpool.tile([P, P], F32, tag="s128", name="s128")
        nc.sync.dma_start(out=s128, in_=sums_dram.rearrange("h (a c) -> (h a) c", c=P))
        r128 = rpool.tile([P, P], F32R, tag="r128", name="r128")
        with nc.allow_low_precision(reason="softmax denominators are O(1); fp32r rounding is fine"):
            nc.vector.reciprocal(out=r128, in_=s128)
        nc.sync.dma_start(out=rcp_dram, in_=r128)
        r16 = rpool.tile([N_HEADS, SEQ], F32R, tag="r16", name="r16")
        nc.sync.dma_start(out=r16, in_=rcp_dram.rearrange("(h a) c -> h (a c)", h=N_HEADS))
        for gg in range(PAIRS):
            pb = spool.tile([P, SEQ], F32, tag="ps", name="psb")
            for n0 in (0, 512):
                nc.tensor.matmul(
                    pb[:, n0:n0 + 512], sel[gg], r16[:, n0:n0 + 512],
                    start=True, stop=True,
                )
            nc.vector.tensor_mul(zt[gg], zt[gg], pb)

    if dbg is not None:
        nc.sync.dma_start(out=dbg["va0"], in_=va[0].bitcast(F32))
        for k in range(DCH):
            nc.sync.dma_start(out=dbg["zt"][k], in_=zt[k].bitcast(F32))

    # ---- phase 4: output projection out[s, d] = zt.T @ O + ob ----
    with tc.tile_pool(name="outsb", bufs=3) as outsb, \
         tc.tile_pool(name="po", bufs=2, space="PSUM") as po:
        for t in range(ST):
            pt = po.tile([P, D_MODEL], F32, tag="po", name="pot")
            for k in range(DCH):
                for n0 in range(0, D_MODEL, 512):
                    nc.tensor.matmul(
                        pt[:, n0:n0 + 512],
                        zt[k][:, t * P:(t + 1) * P],
                        owt[k][:, n0:n0 + 512],
                        start=(k == 0), stop=(k == DCH - 1),
                    )
            ot = outsb.tile([P, D_MODEL], F32, tag="ot", name="ot")
            nc.vector.tensor_add(ot, pt, ob_bc)
            nc.sync.dma_start(out=out[t * P:(t + 1) * P, :], in_=ot)

    for pool in (osb, zt_pool, va_pool, kt_pool, qt_pool, smalls):
        pool.release()


def _get_program():
    if "nc" not in _CACHE:
        _CACHE["nc"] = _build_program()
    return _CACHE["nc"]


def _pack_weights(Qs, Qbs, Ks, Kbs, Vs, Vbs, O, Ob):
    f = np.float32
    qst = np.ascontiguousarray(np.transpose(np.asarray(Qs, f), (1, 0, 2)).reshape(D_MODEL, D_MODEL))
    kst = np.ascontiguousarray(np.transpose(np.asarray(Ks, f), (1, 0, 2)).reshape(D_MODEL, D_MODEL))
    vst = np.zeros((D_MODEL, VTOT), f)
    vb = np.zeros((VTOT,), f)
    Vs = np.asarray(Vs, f)
    Vbs = np.asarray(Vbs, f)
    for h in range(N_HEADS):
        vst[:, h * VW:h * VW + D_HEAD] = Vs[h]
        vb[h * VW:h * VW + D_HEAD] = Vbs[h]
        vb[h * VW + D_HEAD] = 1.0
    ow = np.ascontiguousarray(np.asarray(O, f).reshape(D_MODEL, D_MODEL))
    qbf = np.ascontiguousarray(np.asarray(Qbs, f).reshape(D_MODEL))
    kbf = np.ascontiguousarray(np.asarray(Kbs, f).reshape(D_MODEL))
    obf = np.ascontiguousarray(np.asarray(Ob, f).reshape(D_MODEL))
    return qst, kst, vst, ow, qbf, kbf, vb, obf


def kernel(normalized_resid_pre, encoder_output, Qs, Qbs, Ks, Kbs, Vs, Vbs, O, Ob,
           _trace=False, _trace_kwargs=None):
    nc = _get_program()
    qst, kst, vst, ow, qbf, kbf, vb, obf = _pack_weights(Qs, Qbs, Ks, Kbs, Vs, Vbs, O, Ob)
    enc = np.asarray(encoder_output, np.float32)
    nrp = np.asarray(normalized_resid_pre, np.float32)
    in_maps = []
    for b in range(BATCH):
        in_maps.append({
            "enc": np.ascontiguousarray(enc[b]),
            "nrp": np.ascontiguousarray(nrp[b]),
            "qst": qst, "kst": kst, "vst": vst, "ow": ow,
            "qb": qbf, "kb": kbf, "vb": vb, "ob": obf,
        })
    res = run_bass_kernel_spmd(
        nc, in_maps, list(range(BATCH)),
        trace=_trace, **(_trace_kwargs or {}),
    )
    out = np.stack([res.results[b]["out"] for b in range(BATCH)], axis=0)
    if _trace:
        _CACHE["last_results"] = res
    return out



# revision 4
# speedup vs baseline: 1.5895x; 1.5895x over previous
"""DecoderAttention Bass/Tile kernel for TRN2, batch-parallel over 8 NeuronCores.

Each core handles one batch element:
  q = enc @ Qs + Qbs ; k = enc @ Ks + Kbs ; v = nrp @ Vs + Vbs   (per head)
  scores = q k^T / sqrt(64), causal mask, softmax
  out = (attn @ v) @ O + Ob

v2 design (vs the fp32r baseline):
  - all matmuls in bf16 (PSUM accumulates fp32): enables FWL weight loads and
    1 cyc/row at every moving width; rel err ~3e-3 vs the 2e-2 gate
  - enc/nrp transposed on HOST (numpy .T) -> encT/nrpT DMA'd directly; no
    on-device transpose phase at all
  - scores for the two heads of a pair run CONCURRENTLY via PE row-tiling
    (K=64 stationaries at base partitions 0 and 64 -> tile_position (0,0)
    and (64,0)); both heads' scoresT land in one [128,1024] PSUM tile
  - causal diag masking via gpsimd affine_select zeroing exp output (no PE
    mask matmul)
  - attn@v uses M=128 stationary slices of va (ones-column trick for row
    sums at out row 64; cols 65..127 are junk, ignored) to keep the PE
    array fully occupied for the HAM activity monitor
  - exp for both heads in one ACT call via a 2-region access pattern
  - softmax denominators: DVE row copies -> DRAM -> batched [64,128]
    reciprocal per half, first half overlapped with attention of pairs 4-7;
    normalization broadcast matmuls interleaved as PE filler
"""

import numpy as np
import ml_dtypes

import concourse.bass as bass
import concourse.mybir as mybir
import concourse.tile as tile
from concourse import bacc
from concourse.bass_utils import run_bass_kernel_spmd

N_HEADS, D_MODEL, D_HEAD = 16, 1024, 64
BATCH, SEQ = 8, 1024
P = 128
DCH = D_MODEL // P       # 8 contraction chunks
ST = SEQ // P            # 8 seq tiles
PAIRS = N_HEADS // 2     # 8 head pairs
VW = 65                  # v width per head incl. ones column
VTOT = N_HEADS * VW      # 1040
VPAD = 15 * VW + P + 1   # 1104: last head's 128-wide stationary slice fits
SCALE = 0.125            # 1/sqrt(64)

F32 = mybir.dt.float32
F32R = mybir.dt.float32r
BF16 = mybir.dt.bfloat16
AF = mybir.ActivationFunctionType
BFNP = ml_dtypes.bfloat16

_CACHE = {}

MERGED_EXP = True


def _bcast_row_ap(src, n):
    # DMA access pattern replicating a [n]-element DRAM row to 128 partitions
    return bass.AP(tensor=src.tensor, offset=src.offset, ap=[[0, P], [1, n]])


def _build_program():
    nc = bacc.Bacc("TRN2", target_bir_lowering=False, debug=False, num_devices=8)

    encT = nc.dram_tensor("encT", [D_MODEL, SEQ], BF16, kind="ExternalInput").ap()
    nrpT = nc.dram_tensor("nrpT", [D_MODEL, SEQ], BF16, kind="ExternalInput").ap()
    qwd = nc.dram_tensor("qwd", [D_MODEL, D_MODEL], BF16, kind="ExternalInput").ap()
    kwd = nc.dram_tensor("kwd", [D_MODEL, D_MODEL], BF16, kind="ExternalInput").ap()
    vwd = nc.dram_tensor("vwd", [D_MODEL, VTOT], BF16, kind="ExternalInput").ap()
    owd = nc.dram_tensor("owd", [D_MODEL, D_MODEL], BF16, kind="ExternalInput").ap()
    qb = nc.dram_tensor("qb", [D_MODEL], F32, kind="ExternalInput").ap()
    kb = nc.dram_tensor("kb", [D_MODEL], F32, kind="ExternalInput").ap()
    vb = nc.dram_tensor("vb", [VPAD], F32, kind="ExternalInput").ap()
    ob = nc.dram_tensor("ob", [D_MODEL], F32, kind="ExternalInput").ap()
    out = nc.dram_tensor("out", [SEQ, D_MODEL], F32, kind="ExternalOutput").ap()
    sums_dram = nc.dram_tensor("sums_scratch", [N_HEADS, SEQ], F32).ap()
    rcp_dram = nc.dram_tensor("rcp_scratch", [P, P], F32R).ap()

    with tile.TileContext(nc) as tc:
        _kernel(tc, out, encT, nrpT, qwd, kwd, vwd, owd, qb, kb, vb, ob,
                sums_dram, rcp_dram)
    nc.compile()
    return nc


def _kernel(tc, out, encT, nrpT, qwd, kwd, vwd, owd, qb, kb, vb, ob,
            sums_dram, rcp_dram):
    nc = tc.nc

    # ---- persistent left-side pools ----
    smalls = tc.alloc_tile_pool(name="smalls", bufs=1)
    vb_bc = smalls.tile([P, VTOT], F32, tag="vb_bc", name="vb_bc")
    ob_bc = smalls.tile([P, D_MODEL], F32, tag="ob_bc", name="ob_bc")
    qb_col = smalls.tile([P, PAIRS], F32, tag="qb_col", name="qb_col")
    kb_col = smalls.tile([P, PAIRS], F32, tag="kb_col", name="kb_col")
    s128 = smalls.tile([P, P], F32, tag="s128", name="s128")
    r128 = smalls.tile([P, P], F32R, tag="r128", name="r128")
    r16 = smalls.tile([P, SEQ], F32R, tag="r16", name="r16")
    sel = [smalls.tile([P, P], F32R, tag=f"sel{g}", name=f"sel{g}") for g in range(PAIRS)]
    self_f = smalls.tile([P, P], F32, tag="self_f", name="self_f")

    qt_pool = tc.alloc_tile_pool(name="qt", bufs=1)
    kt_pool = tc.alloc_tile_pool(name="kt", bufs=1)
    va_pool = tc.alloc_tile_pool(name="va", bufs=1)
    zt_pool = tc.alloc_tile_pool(name="zt", bufs=1)
    osb = tc.alloc_tile_pool(name="osb", bufs=1)
    qt = [qt_pool.tile([P, SEQ], BF16, tag=f"qt{g}", name=f"qt{g}") for g in range(PAIRS)]
    kt = [kt_pool.tile([P, SEQ], BF16, tag=f"kt{g}", name=f"kt{g}") for g in range(PAIRS)]
    va = [va_pool.tile([P, VPAD], BF16, tag=f"va{t}", name=f"va{t}") for t in range(ST)]
    zt = [zt_pool.tile([P, SEQ], BF16, tag=f"zt{k}", name=f"zt{k}") for k in range(DCH)]
    owt = [osb.tile([P, D_MODEL], BF16, tag=f"ow{k}", name=f"owt{k}") for k in range(DCH)]

    # ---- right-side transient pools; alloc order = reverse release order ----
    nrp_t_pool = tc.alloc_tile_pool(name="nrpT", bufs=1, side="right")
    vw_pool = tc.alloc_tile_pool(name="vw", bufs=1, side="right")
    enc_t_pool = tc.alloc_tile_pool(name="encT", bufs=1, side="right")
    qw_pool = tc.alloc_tile_pool(name="qw", bufs=1, side="right")
    kw_pool = tc.alloc_tile_pool(name="kw", bufs=1, side="right")
    encS = [enc_t_pool.tile([P, SEQ], BF16, tag=f"e{c}", name=f"encS{c}") for c in range(DCH)]
    nrpS = [nrp_t_pool.tile([P, SEQ], BF16, tag=f"n{c}", name=f"nrpS{c}") for c in range(DCH)]
    qw = [qw_pool.tile([P, D_MODEL], BF16, tag=f"q{c}", name=f"qw{c}") for c in range(DCH)]
    kw = [kw_pool.tile([P, D_MODEL], BF16, tag=f"k{c}", name=f"kw{c}") for c in range(DCH)]
    vw = [vw_pool.tile([P, VTOT], BF16, tag=f"v{c}", name=f"vw{c}") for c in range(DCH)]

    # ---- input DMAs, issued up front in consumption order ----
    # sync queue: activations (enc needed first), then biases
    for c in range(DCH):
        nc.sync.dma_start(out=encS[c], in_=encT[c * P:(c + 1) * P, :])
    for c in range(DCH):
        nc.sync.dma_start(out=nrpS[c], in_=nrpT[c * P:(c + 1) * P, :])
    nc.sync.dma_start(out=qb_col, in_=qb.rearrange("(g p) -> p g", p=P))
    nc.sync.dma_start(out=kb_col, in_=kb.rearrange("(g p) -> p g", p=P))
    nc.sync.dma_start(out=vb_bc, in_=_bcast_row_ap(vb, VTOT))
    nc.sync.dma_start(out=ob_bc, in_=_bcast_row_ap(ob, D_MODEL))
    # scalar queue: weights in use order
    for c in range(DCH):
        nc.scalar.dma_start(out=qw[c], in_=qwd[c * P:(c + 1) * P, :])
    for c in range(DCH):
        nc.scalar.dma_start(out=kw[c], in_=kwd[c * P:(c + 1) * P, :])
    for c in range(DCH):
        nc.scalar.dma_start(out=vw[c], in_=vwd[c * P:(c + 1) * P, :])
    for k in range(DCH):
        nc.scalar.dma_start(out=owt[k], in_=owd[k * P:(k + 1) * P, :])

    # ---- one-time small builds (gpsimd + DVE, off the PE critical path) ----
    # r16 rows 16..127 are read by norm matmuls against zero sel rows: zero them
    nc.gpsimd.memset(r16.bitcast(F32), 0.0)
    # va pad columns (read as junk stationary cols, must be initialized)
    for t in range(ST):
        nc.gpsimd.memset(va[t][:, VTOT:VPAD], 0.0)
    # sel[g][j, p] = 1 where j == 2g + p // 64, zero elsewhere (K=128 padded)
    for g in range(PAIRS):
        nc.gpsimd.memset(self_f, 0.0)
        nc.gpsimd.affine_select(
            out=self_f[0:N_HEADS, :].rearrange("j (a c) -> j a c", a=2),
            in_=self_f[0:N_HEADS, :].rearrange("j (a c) -> j a c", a=2),
            compare_op=mybir.AluOpType.not_equal,
            fill=1.0, base=-2 * g,
            pattern=[[-1, 2], [0, D_HEAD]], channel_multiplier=1,
        )
        nc.vector.tensor_copy(sel[g], self_f)

    # ---- phase 1: q/k projections -> qt/kt [pair-dims 128, seq] bf16 ----
    with tc.tile_pool(name="pproj", bufs=1, space="PSUM") as pproj:
        for wt, bcol, dst in ((qw, qb_col, qt), (kw, kb_col, kt)):
            for n0 in range(0, SEQ, 512):
                ptiles = [pproj.tile([P, 512], F32, tag=f"pp{g}", name=f"pp{g}")
                          for g in range(PAIRS)]
                for c in range(DCH):
                    for g in range(PAIRS):
                        nc.tensor.matmul(
                            ptiles[g],
                            wt[c][:, g * P:(g + 1) * P],
                            encS[c][:, n0:n0 + 512],
                            start=(c == 0), stop=(c == DCH - 1),
                        )
                for g in range(PAIRS):
                    nc.vector.tensor_scalar_add(
                        out=dst[g][:, n0:n0 + 512],
                        in0=ptiles[g],
                        scalar1=bcol[:, g:g + 1],
                    )
    kw_pool.release()
    qw_pool.release()
    enc_t_pool.release()

    # ---- phase 2: v projection -> va [m 128, VTOT] bf16 (ones col via bias) ----
    with tc.tile_pool(name="pv", bufs=2, space="PSUM") as pv:
        for t in range(ST):
            pt = pv.tile([P, VTOT], F32, tag="pv", name="pvt")
            for c in range(DCH):
                for n0 in range(0, VTOT, 512):
                    nw = min(512, VTOT - n0)
                    nc.tensor.matmul(
                        pt[:, n0:n0 + nw],
                        nrpS[c][:, t * P:(t + 1) * P],
                        vw[c][:, n0:n0 + nw],
                        start=(c == 0), stop=(c == DCH - 1),
                    )
            nc.vector.tensor_add(va[t][:, 0:VTOT], pt, vb_bc)
    vw_pool.release()
    nrp_t_pool.release()

    # ---- phase 3: attention ----
    with tc.tile_pool(name="attn", bufs=3) as apool, \
         tc.tile_pool(name="stg", bufs=2) as stg, \
         tc.tile_pool(name="ps_s", bufs=2, space="PSUM") as spool, \
         tc.tile_pool(name="ps_z", bufs=1, space="PSUM") as zpool, \
         tc.tile_pool(name="ps_n", bufs=2, space="PSUM") as npool:

        def norm_pair(g):
            # zt[g] *= recip broadcast: pb[j-dims, q] = sel[g]^T @ r16
            for n0 in range(0, SEQ, 512):
                pb = npool.tile([P, 512], F32, tag="pb", name="pb")
                nc.tensor.matmul(pb, sel[g], r16[:, n0:n0 + 512],
                                 start=True, stop=True, skip_group_check=True)
                nc.vector.tensor_mul(zt[g][:, n0:n0 + 512], zt[g][:, n0:n0 + 512], pb)

        def recip_half(h0):
            # heads h0..h0+7: sums -> [64,128] reciprocal -> r16 rows h0..h0+7
            r0 = h0 * ST
            nc.sync.dma_start(
                out=s128[r0:r0 + 64, :],
                in_=sums_dram[h0:h0 + 8, :].rearrange("h (a c) -> (h a) c", c=P),
            )
            with nc.allow_low_precision(reason="softmax denominators are O(1)"):
                nc.vector.reciprocal(out=r128[r0:r0 + 64, :], in_=s128[r0:r0 + 64, :])
            nc.sync.dma_start(out=rcp_dram[r0:r0 + 64, :], in_=r128[r0:r0 + 64, :])
            nc.sync.dma_start(
                out=r16[h0:h0 + 8, :],
                in_=rcp_dram[r0:r0 + 64, :].rearrange("(h a) c -> h (a c)", h=8),
            )

        for g in range(PAIRS):
            he, ho = 2 * g, 2 * g + 1
            for qh in range(2):
                imax = 4 if qh == 0 else 8
                pz_e = zpool.tile([P, 512], F32, tag="pze", name="pze")
                pz_o = zpool.tile([P, 512], F32, tag="pzo", name="pzo")

                def av_mms(i, ae, cs):
                    nc.tensor.matmul(
                        pz_e[:, cs:512],
                        va[i][:, he * VW:he * VW + P],
                        ae[:, cs:512],
                        start=(i == 0), stop=(i == imax - 1),
                        skip_group_check=True,
                    )
                    nc.tensor.matmul(
                        pz_o[:, cs:512],
                        va[i][:, ho * VW:ho * VW + P],
                        ae[:, 512 + cs:1024],
                        start=(i == 0), stop=(i == imax - 1),
                        skip_group_check=True,
                    )

                pend = None
                for i in range(imax):
                    q0 = i * P
                    cs = max(0, q0 - qh * 512)
                    ps = spool.tile([P, 1024], F32, tag="ps", name="ps")
                    # both heads' scoresT concurrently via PE row tiling
                    nc.tensor.matmul(
                        ps[:, cs:512],
                        kt[g][0:D_HEAD, q0:q0 + P],
                        qt[g][0:D_HEAD, qh * 512 + cs:(qh + 1) * 512],
                        start=True, stop=True, skip_group_check=True,
                    )
                    nc.tensor.matmul(
                        ps[:, 512 + cs:1024],
                        kt[g][D_HEAD:P, q0:q0 + P],
                        qt[g][D_HEAD:P, qh * 512 + cs:(qh + 1) * 512],
                        start=True, stop=True, skip_group_check=True,
                    )
                    ae = apool.tile([P, 1024], BF16, tag="ae", name="ae")
                    if MERGED_EXP:
                        ps3 = ps.rearrange("p (t c) -> p t c", t=2)[:, :, cs:512]
                        ae3 = ae.rearrange("p (t c) -> p t c", t=2)[:, :, cs:512]
                        nc.scalar.activation(out=ae3, in_=ps3, func=AF.Exp,
                                             scale=float(SCALE))
                    else:
                        nc.scalar.activation(out=ae[:, cs:512], in_=ps[:, cs:512],
                                             func=AF.Exp, scale=float(SCALE))
                        nc.scalar.activation(out=ae[:, 512 + cs:1024],
                                             in_=ps[:, 512 + cs:1024],
                                             func=AF.Exp, scale=float(SCALE))
                    if q0 >= qh * 512:
                        # zero strict-upper of the causal diag block post-exp
                        for half in range(2):
                            d = ae[:, half * 512 + cs:half * 512 + cs + P]
                            nc.gpsimd.affine_select(
                                out=d, in_=d,
                                compare_op=mybir.AluOpType.is_ge,
                                fill=0.0, base=0,
                                pattern=[[1, P]], channel_multiplier=-1,
                            )
                    if pend is not None:
                        av_mms(*pend)
                    pend = (i, ae, cs)
                av_mms(*pend)

                # unnormalized z -> zt (bf16); sums row -> DRAM staging
                nc.vector.tensor_copy(zt[g][0:D_HEAD, qh * 512:(qh + 1) * 512],
                                      pz_e[0:D_HEAD, :])
                nc.vector.tensor_copy(zt[g][D_HEAD:P, qh * 512:(qh + 1) * 512],
                                      pz_o[0:D_HEAD, :])
                srow = stg.tile([D_HEAD + 1, 1024], F32, tag="srow", name="srow")
                nc.vector.tensor_copy(srow[D_HEAD:D_HEAD + 1, 0:512],
                                      pz_e[D_HEAD:D_HEAD + 1, :])
                nc.vector.tensor_copy(srow[D_HEAD:D_HEAD + 1, 512:1024],
                                      pz_o[D_HEAD:D_HEAD + 1, :])
                nc.sync.dma_start(
                    out=sums_dram[he:he + 2, qh * 512:(qh + 1) * 512],
                    in_=srow[D_HEAD:D_HEAD + 1, :],
                )

            if g == 3:
                recip_half(0)
            if g >= 4:
                norm_pair(g - 4)
        recip_half(8)
        for g in range(4, PAIRS):
            norm_pair(g)

    # ---- phase 4: output projection out[s, d] = zt.T @ O + ob ----
    with tc.tile_pool(name="outsb", bufs=3) as outsb, \
         tc.tile_pool(name="po", bufs=2, space="PSUM") as po:
        for t in range(ST):
            pt = po.tile([P, D_MODEL], F32, tag="po", name="pot")
            for k in range(DCH):
                for n0 in range(0, D_MODEL, 512):
                    nc.tensor.matmul(
                        pt[:, n0:n0 + 512],
                        zt[k][:, t * P:(t + 1) * P],
                        owt[k][:, n0:n0 + 512],
                        start=(k == 0), stop=(k == DCH - 1),
                    )
            ot = outsb.tile([P, D_MODEL], F32, tag="ot", name="ot")
            nc.vector.tensor_add(ot, pt, ob_bc)
            nc.sync.dma_start(out=out[t * P:(t + 1) * P, :], in_=ot)

    for pool in (osb, zt_pool, va_pool, kt_pool, qt_pool, smalls):
        pool.release()


def _get_program():
    if "nc" not in _CACHE:
        _CACHE["nc"] = _build_program()
    return _CACHE["nc"]


def _pack_weights(Qs, Qbs, Ks, Kbs, Vs, Vbs, O, Ob):
    f = np.float32
    qwd = np.ascontiguousarray(
        np.transpose(np.asarray(Qs, f), (1, 0, 2)).reshape(D_MODEL, D_MODEL)
    ).astype(BFNP)
    kwd = np.ascontiguousarray(
        np.transpose(np.asarray(Ks, f), (1, 0, 2)).reshape(D_MODEL, D_MODEL)
    ).astype(BFNP)
    vwd = np.zeros((D_MODEL, VTOT), f)
    vb = np.zeros((VPAD,), f)
    Vs = np.asarray(Vs, f)
    Vbs = np.asarray(Vbs, f)
    for h in range(N_HEADS):
        vwd[:, h * VW:h * VW + D_HEAD] = Vs[h]
        vb[h * VW:h * VW + D_HEAD] = Vbs[h]
        vb[h * VW + D_HEAD] = 1.0
    vwd = vwd.astype(BFNP)
    owd = np.ascontiguousarray(np.asarray(O, f).reshape(D_MODEL, D_MODEL)).astype(BFNP)
    qbf = np.ascontiguousarray(np.asarray(Qbs, f).reshape(D_MODEL))
    kbf = np.ascontiguousarray(np.asarray(Kbs, f).reshape(D_MODEL))
    obf = np.ascontiguousarray(np.asarray(Ob, f).reshape(D_MODEL))
    return qwd, kwd, vwd, owd, qbf, kbf, vb, obf


def kernel(normalized_resid_pre, encoder_output, Qs, Qbs, Ks, Kbs, Vs, Vbs, O, Ob,
           _trace=False, _trace_kwargs=None):
    nc = _get_program()
    qwd, kwd, vwd, owd, qbf, kbf, vb, obf = _pack_weights(Qs, Qbs, Ks, Kbs, Vs, Vbs, O, Ob)
    enc = np.asarray(encoder_output, np.float32)
    nrp = np.asarray(normalized_resid_pre, np.float32)
    in_maps = []
    for b in range(BATCH):
        in_maps.append({
            "encT": np.ascontiguousarray(enc[b].T).astype(BFNP),
            "nrpT": np.ascontiguousarray(nrp[b].T).astype(BFNP),
            "qwd": qwd, "kwd": kwd, "vwd": vwd, "owd": owd,
            "qb": qbf, "kb": kbf, "vb": vb, "ob": obf,
        })
    res = run_bass_kernel_spmd(
        nc, in_maps, list(range(BATCH)),
        trace=_trace, **(_trace_kwargs or {}),
    )
    out = np.stack([res.results[b]["out"] for b in range(BATCH)], axis=0)
    if _trace:
        _CACHE["last_results"] = res
    return out


# revision 11
# speedup vs baseline: 1.6106x; 1.0133x over previous
"""DecoderAttention Bass/Tile kernel for TRN2, batch-parallel over 8 NeuronCores.

Each core handles one batch element:
  q = enc @ Qs + Qbs ; k = enc @ Ks + Kbs ; v = nrp @ Vs + Vbs   (per head)
  scores = q k^T / sqrt(64), causal mask, softmax
  out = (attn @ v) @ O + Ob

v2 design (vs the fp32r baseline):
  - all matmuls in bf16 (PSUM accumulates fp32): enables FWL weight loads and
    1 cyc/row at every moving width; rel err ~3e-3 vs the 2e-2 gate
  - enc/nrp transposed on HOST (numpy .T) -> encT/nrpT DMA'd directly; no
    on-device transpose phase at all
  - scores for the two heads of a pair run CONCURRENTLY via PE row-tiling
    (K=64 stationaries at base partitions 0 and 64 -> tile_position (0,0)
    and (64,0)); both heads' scoresT land in one [128,1024] PSUM tile
  - causal diag masking via gpsimd affine_select zeroing exp output (no PE
    mask matmul)
  - attn@v uses M=128 stationary slices of va (ones-column trick for row
    sums at out row 64; cols 65..127 are junk, ignored) to keep the PE
    array fully occupied for the HAM activity monitor
  - exp for both heads in one ACT call via a 2-region access pattern
  - softmax denominators: DVE row copies -> DRAM -> batched [64,128]
    reciprocal per half, first half overlapped with attention of pairs 4-7;
    normalization broadcast matmuls interleaved as PE filler
"""

import numpy as np
import ml_dtypes

import concourse.bass as bass
import concourse.mybir as mybir
import concourse.tile as tile
from concourse import bacc
from concourse.bass_utils import run_bass_kernel_spmd

N_HEADS, D_MODEL, D_HEAD = 16, 1024, 64
BATCH, SEQ = 8, 1024
P = 128
DCH = D_MODEL // P       # 8 contraction chunks
ST = SEQ // P            # 8 seq tiles
PAIRS = N_HEADS // 2     # 8 head pairs
VW = 65                  # v width per head incl. ones column
VTOT = N_HEADS * VW      # 1040
VPAD = 15 * VW + P + 1   # 1104: last head's 128-wide stationary slice fits
SCALE = 0.125            # 1/sqrt(64)

F32 = mybir.dt.float32
F32R = mybir.dt.float32r
BF16 = mybir.dt.bfloat16
AF = mybir.ActivationFunctionType
BFNP = ml_dtypes.bfloat16

_CACHE = {}

MERGED_EXP = True


def _bcast_row_ap(src, n):
    # DMA access pattern replicating a [n]-element DRAM row to 128 partitions
    return bass.AP(tensor=src.tensor, offset=src.offset, ap=[[0, P], [1, n]])


def _build_program():
    nc = bacc.Bacc("TRN2", target_bir_lowering=False, debug=False, num_devices=8)

    encT = nc.dram_tensor("encT", [D_MODEL, SEQ], BF16, kind="ExternalInput").ap()
    nrpT = nc.dram_tensor("nrpT", [D_MODEL, SEQ], BF16, kind="ExternalInput").ap()
    qwd = nc.dram_tensor("qwd", [D_MODEL, D_MODEL], BF16, kind="ExternalInput").ap()
    kwd = nc.dram_tensor("kwd", [D_MODEL, D_MODEL], BF16, kind="ExternalInput").ap()
    vwd = nc.dram_tensor("vwd", [D_MODEL, VTOT], BF16, kind="ExternalInput").ap()
    owd = nc.dram_tensor("owd", [D_MODEL, D_MODEL], BF16, kind="ExternalInput").ap()
    qb = nc.dram_tensor("qb", [D_MODEL], F32, kind="ExternalInput").ap()
    kb = nc.dram_tensor("kb", [D_MODEL], F32, kind="ExternalInput").ap()
    vb = nc.dram_tensor("vb", [VPAD], F32, kind="ExternalInput").ap()
    ob = nc.dram_tensor("ob", [D_MODEL], F32, kind="ExternalInput").ap()
    out = nc.dram_tensor("out", [SEQ, D_MODEL], F32, kind="ExternalOutput").ap()
    sums_dram = nc.dram_tensor("sums_scratch", [N_HEADS, SEQ], F32).ap()
    rcp_dram = nc.dram_tensor("rcp_scratch", [P, P], F32R).ap()

    with tile.TileContext(nc) as tc:
        _kernel(tc, out, encT, nrpT, qwd, kwd, vwd, owd, qb, kb, vb, ob,
                sums_dram, rcp_dram)
    nc.compile()
    return nc


def _kernel(tc, out, encT, nrpT, qwd, kwd, vwd, owd, qb, kb, vb, ob,
            sums_dram, rcp_dram):
    nc = tc.nc

    # ---- persistent left-side pools ----
    smalls = tc.alloc_tile_pool(name="smalls", bufs=1)
    vb_bc = smalls.tile([P, VTOT], F32, tag="vb_bc", name="vb_bc")
    ob_bc = smalls.tile([P, D_MODEL], F32, tag="ob_bc", name="ob_bc")
    qb_col = smalls.tile([P, PAIRS], F32, tag="qb_col", name="qb_col")
    kb_col = smalls.tile([P, PAIRS], F32, tag="kb_col", name="kb_col")
    s128 = smalls.tile([P, P], F32, tag="s128", name="s128")
    r128 = smalls.tile([P, P], F32R, tag="r128", name="r128")
    r16 = smalls.tile([P, SEQ], F32R, tag="r16", name="r16")
    sel = [smalls.tile([P, P], F32R, tag=f"sel{g}", name=f"sel{g}") for g in range(PAIRS)]
    self_f = smalls.tile([P, P], F32, tag="self_f", name="self_f")

    qt_pool = tc.alloc_tile_pool(name="qt", bufs=1)
    kt_pool = tc.alloc_tile_pool(name="kt", bufs=1)
    va_pool = tc.alloc_tile_pool(name="va", bufs=1)
    zt_pool = tc.alloc_tile_pool(name="zt", bufs=1)
    osb = tc.alloc_tile_pool(name="osb", bufs=1)
    qt = [qt_pool.tile([P, SEQ], BF16, tag=f"qt{g}", name=f"qt{g}") for g in range(PAIRS)]
    kt = [kt_pool.tile([P, SEQ], BF16, tag=f"kt{g}", name=f"kt{g}") for g in range(PAIRS)]
    va = [va_pool.tile([P, VPAD], BF16, tag=f"va{t}", name=f"va{t}") for t in range(ST)]
    zt = [zt_pool.tile([P, SEQ], BF16, tag=f"zt{k}", name=f"zt{k}") for k in range(DCH)]
    owt = [osb.tile([P, D_MODEL], BF16, tag=f"ow{k}", name=f"owt{k}") for k in range(DCH)]

    # ---- right-side transient pools; alloc order = reverse release order ----
    nrp_t_pool = tc.alloc_tile_pool(name="nrpT", bufs=1, side="right")
    vw_pool = tc.alloc_tile_pool(name="vw", bufs=1, side="right")
    enc_t_pool = tc.alloc_tile_pool(name="encT", bufs=1, side="right")
    qw_pool = tc.alloc_tile_pool(name="qw", bufs=1, side="right")
    kw_pool = tc.alloc_tile_pool(name="kw", bufs=1, side="right")
    encS = [enc_t_pool.tile([P, SEQ], BF16, tag=f"e{c}", name=f"encS{c}") for c in range(DCH)]
    nrpS = [nrp_t_pool.tile([P, SEQ], BF16, tag=f"n{c}", name=f"nrpS{c}") for c in range(DCH)]
    qw = [qw_pool.tile([P, D_MODEL], BF16, tag=f"q{c}", name=f"qw{c}") for c in range(DCH)]
    kw = [kw_pool.tile([P, D_MODEL], BF16, tag=f"k{c}", name=f"kw{c}") for c in range(DCH)]
    vw = [vw_pool.tile([P, VTOT], BF16, tag=f"v{c}", name=f"vw{c}") for c in range(DCH)]

    # ---- input DMAs, issued up front in consumption order ----
    # sync queue: activations (enc needed first), then biases
    for c in range(DCH):
        nc.sync.dma_start(out=encS[c], in_=encT[c * P:(c + 1) * P, :])
    for c in range(DCH):
        nc.sync.dma_start(out=nrpS[c], in_=nrpT[c * P:(c + 1) * P, :])
    nc.sync.dma_start(out=qb_col, in_=qb.rearrange("(g p) -> p g", p=P))
    nc.sync.dma_start(out=kb_col, in_=kb.rearrange("(g p) -> p g", p=P))
    nc.sync.dma_start(out=vb_bc, in_=_bcast_row_ap(vb, VTOT))
    nc.sync.dma_start(out=ob_bc, in_=_bcast_row_ap(ob, D_MODEL))
    # scalar queue: weights in use order (q/k interleaved: k-proj follows q-proj
    # closely; vw before ow since v-proj precedes out-proj)
    for c in range(DCH):
        nc.scalar.dma_start(out=qw[c], in_=qwd[c * P:(c + 1) * P, :])
        nc.scalar.dma_start(out=kw[c], in_=kwd[c * P:(c + 1) * P, :])
    for c in range(DCH):
        nc.scalar.dma_start(out=vw[c], in_=vwd[c * P:(c + 1) * P, :])
    for k in range(DCH):
        nc.scalar.dma_start(out=owt[k], in_=owd[k * P:(k + 1) * P, :])

    # ---- one-time small builds (gpsimd + DVE, off the PE critical path) ----
    # r16 rows 16..127 are read by norm matmuls against zero sel rows: zero them
    nc.gpsimd.memset(r16.bitcast(F32), 0.0)
    # va pad columns (read as junk stationary cols, must be initialized)
    for t in range(ST):
        nc.gpsimd.memset(va[t][:, VTOT:VPAD], 0.0)
    # sel[g][j, p] = 1 where j == 2g + p // 64, zero elsewhere (K=128 padded)
    for g in range(PAIRS):
        nc.gpsimd.memset(self_f, 0.0)
        nc.gpsimd.affine_select(
            out=self_f[0:N_HEADS, :].rearrange("j (a c) -> j a c", a=2),
            in_=self_f[0:N_HEADS, :].rearrange("j (a c) -> j a c", a=2),
            compare_op=mybir.AluOpType.not_equal,
            fill=1.0, base=-2 * g,
            pattern=[[-1, 2], [0, D_HEAD]], channel_multiplier=1,
        )
        nc.vector.tensor_copy(sel[g], self_f)

    # ---- phase 1: q/k projections -> qt/kt [pair-dims 128, seq] bf16 ----
    # pair-groups of 4 -> 4 PSUM banks per group; group B's matmuls overlap
    # group A's bias-add drains
    with tc.tile_pool(name="pproj", bufs=2, space="PSUM") as pproj:
        for wt, bcol, dst in ((qw, qb_col, qt), (kw, kb_col, kt)):
            for n0 in range(0, SEQ, 512):
                for g0 in (0, 4):
                    ptiles = [pproj.tile([P, 512], F32, tag=f"pp{g}", name=f"pp{g}")
                              for g in range(4)]
                    for c in range(DCH):
                        for gi in range(4):
                            nc.tensor.matmul(
                                ptiles[gi],
                                wt[c][:, (g0 + gi) * P:(g0 + gi + 1) * P],
                                encS[c][:, n0:n0 + 512],
                                start=(c == 0), stop=(c == DCH - 1),
                            )
                    for gi in range(4):
                        nc.vector.tensor_scalar_add(
                            out=dst[g0 + gi][:, n0:n0 + 512],
                            in0=ptiles[gi],
                            scalar1=bcol[:, g0 + gi:g0 + gi + 1],
                        )
    kw_pool.release()
    qw_pool.release()
    enc_t_pool.release()

    # ---- phase 2: v projection -> va [m 128, VTOT] bf16 (ones col via bias) ----
    with tc.tile_pool(name="pv", bufs=2, space="PSUM") as pv:
        for t in range(ST):
            pt = pv.tile([P, VTOT], F32, tag="pv", name="pvt")
            for c in range(DCH):
                for n0 in range(0, VTOT, 512):
                    nw = min(512, VTOT - n0)
                    nc.tensor.matmul(
                        pt[:, n0:n0 + nw],
                        nrpS[c][:, t * P:(t + 1) * P],
                        vw[c][:, n0:n0 + nw],
                        start=(c == 0), stop=(c == DCH - 1),
                    )
            nc.vector.tensor_add(va[t][:, 0:VTOT], pt, vb_bc)
    vw_pool.release()
    nrp_t_pool.release()

    # ---- phase 3: attention ----
    with tc.tile_pool(name="attn", bufs=3) as apool, \
         tc.tile_pool(name="stg", bufs=2) as stg, \
         tc.tile_pool(name="ps_s", bufs=2, space="PSUM") as spool, \
         tc.tile_pool(name="ps_z", bufs=2, space="PSUM") as zpool:

        def norm_pair(g):
            # zt[g] *= recip broadcast: pb[j-dims, q] = sel[g]^T @ r16
            # (pb shares the spool "ps" slots; uses cols 0:512)
            for n0 in range(0, SEQ, 512):
                pb = spool.tile([P, 1024], F32, tag="ps", name="pb")
                nc.tensor.matmul(pb[:, 0:512], sel[g], r16[:, n0:n0 + 512],
                                 start=True, stop=True, skip_group_check=True)
                nc.vector.tensor_mul(zt[g][:, n0:n0 + 512], zt[g][:, n0:n0 + 512],
                                     pb[:, 0:512])

        def recip_chunk(h0, nh):
            # heads h0..h0+nh: sums -> [8*nh,128] reciprocal -> r16 rows
            r0 = h0 * ST
            nr = nh * ST
            nc.sync.dma_start(
                out=s128[r0:r0 + nr, :],
                in_=sums_dram[h0:h0 + nh, :].rearrange("h (a c) -> (h a) c", c=P),
            )
            with nc.allow_low_precision(reason="softmax denominators are O(1)"):
                nc.vector.reciprocal(out=r128[r0:r0 + nr, :], in_=s128[r0:r0 + nr, :])
            nc.sync.dma_start(out=rcp_dram[r0:r0 + nr, :], in_=r128[r0:r0 + nr, :])
            nc.sync.dma_start(
                out=r16[h0:h0 + nh, :],
                in_=rcp_dram[r0:r0 + nr, :].rearrange("(h a) c -> h (a c)", h=nh),
            )

        for g in range(PAIRS):
            he, ho = 2 * g, 2 * g + 1
            for qh in range(2):
                imax = 4 if qh == 0 else 8
                pz_e = zpool.tile([P, 512], F32, tag="pze", name="pze")
                pz_o = zpool.tile([P, 512], F32, tag="pzo", name="pzo")

                def av_mms(i, ae, cs):
                    nc.tensor.matmul(
                        pz_e[:, cs:512],
                        va[i][:, he * VW:he * VW + P],
                        ae[:, cs:512],
                        start=(i == 0), stop=(i == imax - 1),
                        skip_group_check=True,
                    )
                    nc.tensor.matmul(
                        pz_o[:, cs:512],
                        va[i][:, ho * VW:ho * VW + P],
                        ae[:, 512 + cs:1024],
                        start=(i == 0), stop=(i == imax - 1),
                        skip_group_check=True,
                    )

                pend = []
                for i in range(imax):
                    q0 = i * P
                    cs = max(0, q0 - qh * 512)
                    ps = spool.tile([P, 1024], F32, tag="ps", name="ps")
                    # both heads' scoresT concurrently via PE row tiling
                    nc.tensor.matmul(
                        ps[:, cs:512],
                        kt[g][0:D_HEAD, q0:q0 + P],
                        qt[g][0:D_HEAD, qh * 512 + cs:(qh + 1) * 512],
                        start=True, stop=True, skip_group_check=True,
                    )
                    nc.tensor.matmul(
                        ps[:, 512 + cs:1024],
                        kt[g][D_HEAD:P, q0:q0 + P],
                        qt[g][D_HEAD:P, qh * 512 + cs:(qh + 1) * 512],
                        start=True, stop=True, skip_group_check=True,
                    )
                    ae = apool.tile([P, 1024], BF16, tag="ae", name="ae")
                    if MERGED_EXP:
                        ps3 = ps.rearrange("p (t c) -> p t c", t=2)[:, :, cs:512]
                        ae3 = ae.rearrange("p (t c) -> p t c", t=2)[:, :, cs:512]
                        nc.scalar.activation(out=ae3, in_=ps3, func=AF.Exp,
                                             scale=float(SCALE))
                    else:
                        nc.scalar.activation(out=ae[:, cs:512], in_=ps[:, cs:512],
                                             func=AF.Exp, scale=float(SCALE))
                        nc.scalar.activation(out=ae[:, 512 + cs:1024],
                                             in_=ps[:, 512 + cs:1024],
                                             func=AF.Exp, scale=float(SCALE))
                    if q0 >= qh * 512:
                        # zero strict-upper of the causal diag block post-exp
                        for half in range(2):
                            d = ae[:, half * 512 + cs:half * 512 + cs + P]
                            nc.gpsimd.affine_select(
                                out=d, in_=d,
                                compare_op=mybir.AluOpType.is_ge,
                                fill=0.0, base=0,
                                pattern=[[1, P]], channel_multiplier=-1,
                            )
                    # av lags two steps behind so exp latency never stalls the PE
                    pend.append((i, ae, cs))
                    if len(pend) > 2:
                        av_mms(*pend.pop(0))
                for item in pend:
                    av_mms(*item)

                # unnormalized z -> zt (bf16); sums row -> DRAM staging
                nc.vector.tensor_copy(zt[g][0:D_HEAD, qh * 512:(qh + 1) * 512],
                                      pz_e[0:D_HEAD, :])
                nc.vector.tensor_copy(zt[g][D_HEAD:P, qh * 512:(qh + 1) * 512],
                                      pz_o[0:D_HEAD, :])
                srow = stg.tile([D_HEAD + 1, 1024], F32, tag="srow", name="srow")
                nc.vector.tensor_copy(srow[D_HEAD:D_HEAD + 1, 0:512],
                                      pz_e[D_HEAD:D_HEAD + 1, :])
                nc.vector.tensor_copy(srow[D_HEAD:D_HEAD + 1, 512:1024],
                                      pz_o[D_HEAD:D_HEAD + 1, :])
                nc.sync.dma_start(
                    out=sums_dram[he:he + 2, qh * 512:(qh + 1) * 512],
                    in_=srow[D_HEAD:D_HEAD + 1, :],
                )

            if g == 3:
                recip_chunk(0, 8)
            elif g == 4:
                norm_pair(0)
            elif g == 5:
                norm_pair(1)
                recip_chunk(8, 4)
            elif g == 6:
                norm_pair(2)
                norm_pair(4)
            elif g == 7:
                norm_pair(3)
                norm_pair(5)
                recip_chunk(12, 4)
                norm_pair(6)
                norm_pair(7)

    # ---- phase 4: output projection out[s, d] = zt.T @ O + ob ----
    # chunks 6,7 (the last-normalized pairs) accumulate last, and two t-tiles
    # are in flight so their k=0..5 matmuls hide the tail normalization
    with tc.tile_pool(name="outsb", bufs=3) as outsb, \
         tc.tile_pool(name="po", bufs=2, space="PSUM") as po:
        for t0 in range(0, ST, 2):
            pts = {}
            for t in (t0, t0 + 1):
                pts[t] = po.tile([P, D_MODEL], F32, tag=f"po{t % 2}", name="pot")
                for k in range(6):
                    for n0 in range(0, D_MODEL, 512):
                        nc.tensor.matmul(
                            pts[t][:, n0:n0 + 512],
                            zt[k][:, t * P:(t + 1) * P],
                            owt[k][:, n0:n0 + 512],
                            start=(k == 0), stop=False,
                            skip_group_check=True,
                        )
            for t in (t0, t0 + 1):
                for k in (6, 7):
                    for n0 in range(0, D_MODEL, 512):
                        nc.tensor.matmul(
                            pts[t][:, n0:n0 + 512],
                            zt[k][:, t * P:(t + 1) * P],
                            owt[k][:, n0:n0 + 512],
                            start=False, stop=(k == DCH - 1),
                            skip_group_check=True,
                        )
                ot = outsb.tile([P, D_MODEL], F32, tag="ot", name="ot")
                nc.vector.tensor_add(ot, pts[t], ob_bc)
                nc.sync.dma_start(out=out[t * P:(t + 1) * P, :], in_=ot)

    for pool in (osb, zt_pool, va_pool, kt_pool, qt_pool, smalls):
        pool.release()


def _get_program():
    if "nc" not in _CACHE:
        _CACHE["nc"] = _build_program()
    return _CACHE["nc"]


def _pack_weights(Qs, Qbs, Ks, Kbs, Vs, Vbs, O, Ob):
    f = np.float32
    qwd = np.ascontiguousarray(
        np.transpose(np.asarray(Qs, f), (1, 0, 2)).reshape(D_MODEL, D_MODEL)
    ).astype(BFNP)
    kwd = np.ascontiguousarray(
        np.transpose(np.asarray(Ks, f), (1, 0, 2)).reshape(D_MODEL, D_MODEL)
    ).astype(BFNP)
    vwd = np.zeros((D_MODEL, VTOT), f)
    vb = np.zeros((VPAD,), f)
    Vs = np.asarray(Vs, f)
    Vbs = np.asarray(Vbs, f)
    for h in range(N_HEADS):
        vwd[:, h * VW:h * VW + D_HEAD] = Vs[h]
        vb[h * VW:h * VW + D_HEAD] = Vbs[h]
        vb[h * VW + D_HEAD] = 1.0
    vwd = vwd.astype(BFNP)
    owd = np.ascontiguousarray(np.asarray(O, f).reshape(D_MODEL, D_MODEL)).astype(BFNP)
    qbf = np.ascontiguousarray(np.asarray(Qbs, f).reshape(D_MODEL))
    kbf = np.ascontiguousarray(np.asarray(Kbs, f).reshape(D_MODEL))
    obf = np.ascontiguousarray(np.asarray(Ob, f).reshape(D_MODEL))
    return qwd, kwd, vwd, owd, qbf, kbf, vb, obf


def kernel(normalized_resid_pre, encoder_output, Qs, Qbs, Ks, Kbs, Vs, Vbs, O, Ob,
           _trace=False, _trace_kwargs=None):
    nc = _get_program()
    qwd, kwd, vwd, owd, qbf, kbf, vb, obf = _pack_weights(Qs, Qbs, Ks, Kbs, Vs, Vbs, O, Ob)
    enc = np.asarray(encoder_output, np.float32)
    nrp = np.asarray(normalized_resid_pre, np.float32)
    in_maps = []
    for b in range(BATCH):
        in_maps.append({
            "encT": np.ascontiguousarray(enc[b].T).astype(BFNP),
            "nrpT": np.ascontiguousarray(nrp[b].T).astype(BFNP),
            "qwd": qwd, "kwd": kwd, "vwd": vwd, "owd": owd,
            "qb": qbf, "kb": kbf, "vb": vb, "ob": obf,
        })
    res = run_bass_kernel_spmd(
        nc, in_maps, list(range(BATCH)),
        trace=_trace, **(_trace_kwargs or {}),
    )
    out = np.stack([res.results[b]["out"] for b in range(BATCH)], axis=0)
    if _trace:
        _CACHE["last_results"] = res
    return out


# revision 23
# speedup vs baseline: 1.7811x; 1.1058x over previous
"""DecoderAttention Bass/Tile kernel for TRN2, batch-parallel over 8 NeuronCores.

Each core handles one batch element:
  q = enc @ Qs + Qbs ; k = enc @ Ks + Kbs ; v = nrp @ Vs + Vbs   (per head)
  scores = q k^T / sqrt(64), causal mask, softmax
  out = (attn @ v) @ O + Ob

v2 design (vs the fp32r baseline):
  - all matmuls in bf16 (PSUM accumulates fp32): enables FWL weight loads and
    1 cyc/row at every moving width; rel err ~3e-3 vs the 2e-2 gate
  - enc/nrp transposed on HOST (numpy .T) -> encT/nrpT DMA'd directly; no
    on-device transpose phase at all
  - scores for the two heads of a pair run CONCURRENTLY via PE row-tiling
    (K=64 stationaries at base partitions 0 and 64 -> tile_position (0,0)
    and (64,0)); both heads' scoresT land in one [128,1024] PSUM tile
  - causal diag masking via gpsimd affine_select zeroing exp output (no PE
    mask matmul)
  - attn@v uses M=128 stationary slices of va (ones-column trick for row
    sums at out row 64; cols 65..127 are junk, ignored) to keep the PE
    array fully occupied for the HAM activity monitor
  - exp for both heads in one ACT call via a 2-region access pattern
  - softmax denominators: DVE row copies -> DRAM -> batched [64,128]
    reciprocal per half, first half overlapped with attention of pairs 4-7;
    normalization broadcast matmuls interleaved as PE filler
"""

import numpy as np
import ml_dtypes

import concourse.bass as bass
import concourse.mybir as mybir
import concourse.tile as tile
from concourse import bacc
from concourse.bass_utils import run_bass_kernel_spmd

N_HEADS, D_MODEL, D_HEAD = 16, 1024, 64
BATCH, SEQ = 8, 1024
P = 128
DCH = D_MODEL // P       # 8 contraction chunks
ST = SEQ // P            # 8 seq tiles
PAIRS = N_HEADS // 2     # 8 head pairs
VW = 65                  # v width per head incl. ones column
VTOT = N_HEADS * VW      # 1040
VPAD = 15 * VW + P + 1   # 1104: last head's 128-wide stationary slice fits
SCALE = 0.125            # 1/sqrt(64)

F32 = mybir.dt.float32
F32R = mybir.dt.float32r
BF16 = mybir.dt.bfloat16
AF = mybir.ActivationFunctionType
BFNP = ml_dtypes.bfloat16

_CACHE = {}

MERGED_EXP = True


def _bcast_row_ap(src, n):
    # DMA access pattern replicating a [n]-element DRAM row to 128 partitions
    return bass.AP(tensor=src.tensor, offset=src.offset, ap=[[0, P], [1, n]])


def _build_program():
    nc = bacc.Bacc("TRN2", target_bir_lowering=False, debug=False, num_devices=8)

    encT = nc.dram_tensor("encT", [D_MODEL, SEQ], BF16, kind="ExternalInput").ap()
    nrpT = nc.dram_tensor("nrpT", [D_MODEL, SEQ], BF16, kind="ExternalInput").ap()
    qwd = nc.dram_tensor("qwd", [D_MODEL, D_MODEL], BF16, kind="ExternalInput").ap()
    kwd = nc.dram_tensor("kwd", [D_MODEL, D_MODEL], BF16, kind="ExternalInput").ap()
    vwd = nc.dram_tensor("vwd", [D_MODEL, VTOT], BF16, kind="ExternalInput").ap()
    owd = nc.dram_tensor("owd", [D_MODEL, D_MODEL], BF16, kind="ExternalInput").ap()
    qb = nc.dram_tensor("qb", [D_MODEL], F32, kind="ExternalInput").ap()
    kb = nc.dram_tensor("kb", [D_MODEL], F32, kind="ExternalInput").ap()
    vb = nc.dram_tensor("vb", [VPAD], F32, kind="ExternalInput").ap()
    ob = nc.dram_tensor("ob", [D_MODEL], F32, kind="ExternalInput").ap()
    out = nc.dram_tensor("out", [SEQ, D_MODEL], F32, kind="ExternalOutput").ap()

    with tile.TileContext(nc) as tc:
        _kernel(tc, out, encT, nrpT, qwd, kwd, vwd, owd, qb, kb, vb, ob)
    nc.compile()
    return nc


def _kernel(tc, out, encT, nrpT, qwd, kwd, vwd, owd, qb, kb, vb, ob):
    nc = tc.nc

    # ---- persistent left-side pools ----
    smalls = tc.alloc_tile_pool(name="smalls", bufs=1)
    vb_bc = smalls.tile([P, VTOT], F32, tag="vb_bc", name="vb_bc")
    ob_bc = smalls.tile([P, D_MODEL], F32, tag="ob_bc", name="ob_bc")
    qb_col = smalls.tile([P, PAIRS], F32, tag="qb_col", name="qb_col")
    kb_col = smalls.tile([P, PAIRS], F32, tag="kb_col", name="kb_col")
    s128 = smalls.tile([P, P], F32, tag="s128", name="s128")
    r128 = smalls.tile([P, P], F32R, tag="r128", name="r128")
    r16 = smalls.tile([P, SEQ], F32R, tag="r16", name="r16")
    sel = [smalls.tile([P, P], F32R, tag=f"sel{g}", name=f"sel{g}") for g in range(PAIRS)]
    self_f = smalls.tile([P, P], F32, tag="self_f", name="self_f")

    qt_pool = tc.alloc_tile_pool(name="qt", bufs=1)
    kt_pool = tc.alloc_tile_pool(name="kt", bufs=1)
    va_pool = tc.alloc_tile_pool(name="va", bufs=1)
    zt_pool = tc.alloc_tile_pool(name="zt", bufs=1)
    osb = tc.alloc_tile_pool(name="osb", bufs=1)
    qt = [qt_pool.tile([P, SEQ], BF16, tag=f"qt{g}", name=f"qt{g}") for g in range(PAIRS)]
    kt = [kt_pool.tile([P, SEQ], BF16, tag=f"kt{g}", name=f"kt{g}") for g in range(PAIRS)]
    va = [va_pool.tile([P, VPAD], BF16, tag=f"va{t}", name=f"va{t}") for t in range(ST)]
    zt = [zt_pool.tile([P, SEQ], BF16, tag=f"zt{k}", name=f"zt{k}") for k in range(DCH)]
    owt = [osb.tile([P, D_MODEL], BF16, tag=f"ow{k}", name=f"owt{k}") for k in range(DCH)]

    # ---- right-side transient pools; alloc order = reverse release order ----
    # encS/qw/kw live through attention (q/k filler projections); nrpS/vw die
    # after the v projection
    enc_t_pool = tc.alloc_tile_pool(name="encT", bufs=1, side="right")
    qw_pool = tc.alloc_tile_pool(name="qw", bufs=1, side="right")
    kw_pool = tc.alloc_tile_pool(name="kw", bufs=1, side="right")
    nrp_t_pool = tc.alloc_tile_pool(name="nrpT", bufs=1, side="right")
    vw_pool = tc.alloc_tile_pool(name="vw", bufs=1, side="right")
    encS = [enc_t_pool.tile([P, SEQ], BF16, tag=f"e{c}", name=f"encS{c}") for c in range(DCH)]
    nrpS = [nrp_t_pool.tile([P, SEQ], BF16, tag=f"n{c}", name=f"nrpS{c}") for c in range(DCH)]
    qw = [qw_pool.tile([P, D_MODEL], BF16, tag=f"q{c}", name=f"qw{c}") for c in range(DCH)]
    kw = [kw_pool.tile([P, D_MODEL], BF16, tag=f"k{c}", name=f"kw{c}") for c in range(DCH)]
    vw = [vw_pool.tile([P, VTOT], BF16, tag=f"v{c}", name=f"vw{c}") for c in range(DCH)]

    # ---- input DMAs, issued up front in consumption order ----
    # sync queue: activations (enc needed first), then biases
    for c in range(DCH):
        nc.sync.dma_start(out=encS[c], in_=encT[c * P:(c + 1) * P, :])
    for c in range(DCH):
        nc.sync.dma_start(out=nrpS[c], in_=nrpT[c * P:(c + 1) * P, :])
    nc.sync.dma_start(out=qb_col, in_=qb.rearrange("(g p) -> p g", p=P))
    nc.sync.dma_start(out=kb_col, in_=kb.rearrange("(g p) -> p g", p=P))
    nc.sync.dma_start(out=vb_bc, in_=_bcast_row_ap(vb, VTOT))
    nc.sync.dma_start(out=ob_bc, in_=_bcast_row_ap(ob, D_MODEL))
    # scalar queue: weights in use order (q/k interleaved: k-proj follows q-proj
    # closely; vw before ow since v-proj precedes out-proj)
    for c in range(DCH):
        nc.scalar.dma_start(out=qw[c], in_=qwd[c * P:(c + 1) * P, :])
        nc.scalar.dma_start(out=kw[c], in_=kwd[c * P:(c + 1) * P, :])
    for c in range(DCH):
        nc.scalar.dma_start(out=vw[c], in_=vwd[c * P:(c + 1) * P, :])
    for k in range(DCH):
        nc.scalar.dma_start(out=owt[k], in_=owd[k * P:(k + 1) * P, :])

    # ---- one-time small builds (gpsimd + DVE, off the PE critical path) ----
    # r16 rows 16..127 are read by norm matmuls against zero sel rows: zero them
    nc.gpsimd.memset(r16.bitcast(F32), 0.0)
    # va pad columns (read as junk stationary cols, must be initialized)
    for t in range(ST):
        nc.gpsimd.memset(va[t][:, VTOT:VPAD], 0.0)
    # sel[g][j, p] = 1 where j == 2g + p // 64, zero elsewhere (K=128 padded)
    for g in range(PAIRS):
        nc.gpsimd.memset(self_f, 0.0)
        nc.gpsimd.affine_select(
            out=self_f[0:N_HEADS, :].rearrange("j (a c) -> j a c", a=2),
            in_=self_f[0:N_HEADS, :].rearrange("j (a c) -> j a c", a=2),
            compare_op=mybir.AluOpType.not_equal,
            fill=1.0, base=-2 * g,
            pattern=[[-1, 2], [0, D_HEAD]], channel_multiplier=1,
        )
        nc.vector.tensor_copy(sel[g], self_f)

    # ---- phase 1: q/k projections for pairs 0,1 only ----
    # (pairs 2-7 are emitted later as PE filler inside the attention loop)
    with tc.tile_pool(name="pproj", bufs=2, space="PSUM") as pproj:
        for g in (0, 1):
            for wt, bcol, dst in ((qw, qb_col, qt), (kw, kb_col, kt)):
                for n0 in range(0, SEQ, 512):
                    pp = pproj.tile([P, 512], F32, tag="pp", name="pp")
                    for c in range(DCH):
                        nc.tensor.matmul(
                            pp,
                            wt[c][:, g * P:(g + 1) * P],
                            encS[c][:, n0:n0 + 512],
                            start=(c == 0), stop=(c == DCH - 1),
                        )
                    nc.vector.tensor_scalar_add(
                        out=dst[g][:, n0:n0 + 512],
                        in0=pp,
                        scalar1=bcol[:, g:g + 1],
                    )

    # ---- phase 2: v projection -> va [m 128, VTOT] bf16 (ones col via bias) ----
    with tc.tile_pool(name="pv", bufs=2, space="PSUM") as pv:
        for t in range(ST):
            pt = pv.tile([P, VTOT], F32, tag="pv", name="pvt")
            for c in range(DCH):
                for n0 in range(0, VTOT, 512):
                    nw = min(512, VTOT - n0)
                    nc.tensor.matmul(
                        pt[:, n0:n0 + nw],
                        nrpS[c][:, t * P:(t + 1) * P],
                        vw[c][:, n0:n0 + nw],
                        start=(c == 0), stop=(c == DCH - 1),
                    )
            nc.vector.tensor_add(va[t][:, 0:VTOT], pt, vb_bc)
    vw_pool.release()
    nrp_t_pool.release()
    # note: encS/qw/kw stay alive for the in-attention filler projections

    # ---- phase 3: attention ----
    norm_todo = {}

    def make_norm_pair(spool_tile):
        def norm_pair(g):
            # zt[g] *= recip broadcast: pb[j-dims, q] = sel[g]^T @ r16
            for n0 in range(0, SEQ, 512):
                pb = spool_tile()
                nc.tensor.matmul(pb, sel[g], r16[:, n0:n0 + 512],
                                 start=True, stop=True, skip_group_check=True)
                nc.vector.tensor_mul(zt[g][:, n0:n0 + 512], zt[g][:, n0:n0 + 512], pb)
        return norm_pair

    def recip_chunk(h0, nh):
        # heads h0..h0+nh: [8*nh,128] reciprocal -> r16 rows (all SBUF-local)
        r0 = h0 * ST
        nr = nh * ST
        with nc.allow_low_precision(reason="softmax denominators are O(1)"):
            nc.vector.reciprocal(out=r128[r0:r0 + nr, :], in_=s128[r0:r0 + nr, :])
        nc.sync.dma_start(out=r16[h0:h0 + nh, :], in_=r128[r0:r0 + nr, :])

    with tc.tile_pool(name="attn", bufs=3) as apool, \
         tc.tile_pool(name="stg", bufs=2) as stg, \
         tc.tile_pool(name="ps_s", bufs=2, space="PSUM") as spool, \
         tc.tile_pool(name="ps_z", bufs=1, space="PSUM") as zpool, \
         tc.tile_pool(name="ps_f", bufs=2, space="PSUM") as qkfill:

        norm_pair = make_norm_pair(
            lambda: spool.tile([P, 1024], F32, tag="ps", name="pb")[:, 0:512])

        def filler_groups(tg):
            # q/k projection for pair tg, split into 4 PE work groups that
            # slot into attention's dependency-stall windows
            groups = []
            for wt, bcol, dst in ((qw, qb_col, qt), (kw, kb_col, kt)):
                for n0 in (0, 512):
                    def mk(wt=wt, bcol=bcol, dst=dst, n0=n0):
                        pp = qkfill.tile([P, 512], F32, tag="qkf", name="qkf")
                        for c in range(DCH):
                            nc.tensor.matmul(
                                pp,
                                wt[c][:, tg * P:(tg + 1) * P],
                                encS[c][:, n0:n0 + 512],
                                start=(c == 0), stop=(c == DCH - 1),
                                skip_group_check=True,
                            )
                        nc.vector.tensor_scalar_add(
                            out=dst[tg][:, n0:n0 + 512], in0=pp,
                            scalar1=bcol[:, tg:tg + 1],
                        )
                    groups.append(mk)
            return groups

        for g in range(PAIRS):
            he, ho = 2 * g, 2 * g + 1
            fill = filler_groups(g + 2) if 2 <= g + 2 < PAIRS else []
            step_no = 0
            for qh in range(2):
                imax = 4 if qh == 0 else 8
                pz_e = zpool.tile([P, 512], F32, tag="pze", name="pze")
                pz_o = zpool.tile([P, 512], F32, tag="pzo", name="pzo")

                def av_mms(i, ae, cs):
                    nc.tensor.matmul(
                        pz_e[:, cs:512],
                        va[i][:, he * VW:he * VW + P],
                        ae[:, cs:512],
                        start=(i == 0), stop=(i == imax - 1),
                        skip_group_check=True,
                    )
                    nc.tensor.matmul(
                        pz_o[:, cs:512],
                        va[i][:, ho * VW:ho * VW + P],
                        ae[:, 512 + cs:1024],
                        start=(i == 0), stop=(i == imax - 1),
                        skip_group_check=True,
                    )

                pend = []
                for i in range(imax):
                    q0 = i * P
                    cs = max(0, q0 - qh * 512)
                    ps = spool.tile([P, 1024], F32, tag="ps", name="ps")
                    # both heads' scoresT concurrently via PE row tiling
                    nc.tensor.matmul(
                        ps[:, cs:512],
                        kt[g][0:D_HEAD, q0:q0 + P],
                        qt[g][0:D_HEAD, qh * 512 + cs:(qh + 1) * 512],
                        start=True, stop=True, skip_group_check=True,
                    )
                    nc.tensor.matmul(
                        ps[:, 512 + cs:1024],
                        kt[g][D_HEAD:P, q0:q0 + P],
                        qt[g][D_HEAD:P, qh * 512 + cs:(qh + 1) * 512],
                        start=True, stop=True, skip_group_check=True,
                    )
                    ae = apool.tile([P, 1024], BF16, tag="ae", name="ae")
                    if MERGED_EXP:
                        ps3 = ps.rearrange("p (t c) -> p t c", t=2)[:, :, cs:512]
                        ae3 = ae.rearrange("p (t c) -> p t c", t=2)[:, :, cs:512]
                        nc.scalar.activation(out=ae3, in_=ps3, func=AF.Exp,
                                             scale=float(SCALE))
                    else:
                        nc.scalar.activation(out=ae[:, cs:512], in_=ps[:, cs:512],
                                             func=AF.Exp, scale=float(SCALE))
                        nc.scalar.activation(out=ae[:, 512 + cs:1024],
                                             in_=ps[:, 512 + cs:1024],
                                             func=AF.Exp, scale=float(SCALE))
                    if q0 >= qh * 512:
                        # zero strict-upper of the causal diag block post-exp
                        for half in range(2):
                            d = ae[:, half * 512 + cs:half * 512 + cs + P]
                            nc.gpsimd.affine_select(
                                out=d, in_=d,
                                compare_op=mybir.AluOpType.is_ge,
                                fill=0.0, base=0,
                                pattern=[[1, P]], channel_multiplier=-1,
                            )
                    # av lags two steps behind so exp latency never stalls the PE
                    pend.append((i, ae, cs))
                    if len(pend) > 2:
                        av_mms(*pend.pop(0))
                    if step_no in (1, 4, 7, 10) and fill:
                        fill.pop(0)()
                    step_no += 1
                for item in pend:
                    av_mms(*item)

                # unnormalized z -> zt (bf16); sums row -> DRAM staging
                nc.vector.tensor_copy(zt[g][0:D_HEAD, qh * 512:(qh + 1) * 512],
                                      pz_e[0:D_HEAD, :])
                nc.vector.tensor_copy(zt[g][D_HEAD:P, qh * 512:(qh + 1) * 512],
                                      pz_o[0:D_HEAD, :])
                srow = stg.tile([D_HEAD + 1, 1024], F32, tag="srow", name="srow")
                nc.vector.tensor_copy(srow[D_HEAD:D_HEAD + 1, 0:512],
                                      pz_e[D_HEAD:D_HEAD + 1, :])
                nc.vector.tensor_copy(srow[D_HEAD:D_HEAD + 1, 512:1024],
                                      pz_o[D_HEAD:D_HEAD + 1, :])
                # scatter sums into the [128,128] reciprocal layout (SBUF→SBUF)
                nc.sync.dma_start(
                    out=s128[he * ST + qh * 4:he * ST + qh * 4 + 4, :],
                    in_=srow[D_HEAD:D_HEAD + 1, 0:512],
                )
                nc.sync.dma_start(
                    out=s128[ho * ST + qh * 4:ho * ST + qh * 4 + 4, :],
                    in_=srow[D_HEAD:D_HEAD + 1, 512:1024],
                )

            if g == 3:
                recip_chunk(0, 8)
            elif g == 4:
                norm_pair(0)
            elif g == 5:
                norm_pair(1)
                recip_chunk(8, 4)
            elif g == 6:
                norm_pair(2)
                norm_pair(4)
            elif g == 7:
                norm_pair(3)
                norm_pair(5)
                recip_chunk(12, 4)
                # pairs 6,7 normalize inside the out-projection phase, hidden
                # behind its chunk-0..5 accumulation

    # ---- phase 4: output projection out[s, d] = zt.T @ O + ob ----
    kw_pool.release()
    qw_pool.release()
    enc_t_pool.release()

    # chunks 6,7 (the last-normalized pairs) accumulate last, and two t-tiles
    # are in flight so their k=0..5 matmuls hide the tail normalization
    with tc.tile_pool(name="outsb", bufs=3) as outsb, \
         tc.tile_pool(name="po", bufs=1, space="PSUM") as po, \
         tc.tile_pool(name="pn", bufs=2, space="PSUM") as pnorm:
        tail_norm = make_norm_pair(
            lambda: pnorm.tile([P, 512], F32, tag="pn", name="pn"))
        for t0 in range(0, ST, 2):
            pts = {}
            for t in (t0, t0 + 1):
                pts[t] = po.tile([P, D_MODEL], F32, tag=f"po{t % 2}", name="pot")
                for k in range(6):
                    for n0 in range(0, D_MODEL, 512):
                        nc.tensor.matmul(
                            pts[t][:, n0:n0 + 512],
                            zt[k][:, t * P:(t + 1) * P],
                            owt[k][:, n0:n0 + 512],
                            start=(k == 0), stop=False,
                            skip_group_check=True,
                        )
            if t0 == 0:
                tail_norm(6)
                tail_norm(7)
            for t in (t0, t0 + 1):
                for k in (6, 7):
                    for n0 in range(0, D_MODEL, 512):
                        nc.tensor.matmul(
                            pts[t][:, n0:n0 + 512],
                            zt[k][:, t * P:(t + 1) * P],
                            owt[k][:, n0:n0 + 512],
                            start=False, stop=(k == DCH - 1),
                            skip_group_check=True,
                        )
                ot = outsb.tile([P, D_MODEL], F32, tag="ot", name="ot")
                nc.vector.tensor_add(ot, pts[t], ob_bc)
                nc.sync.dma_start(out=out[t * P:(t + 1) * P, :], in_=ot)

    for pool in (osb, zt_pool, va_pool, kt_pool, qt_pool, smalls):
        pool.release()


def _get_program():
    if "nc" not in _CACHE:
        _CACHE["nc"] = _build_program()
    return _CACHE["nc"]


def _pack_weights(Qs, Qbs, Ks, Kbs, Vs, Vbs, O, Ob):
    f = np.float32
    qwd = np.ascontiguousarray(
        np.transpose(np.asarray(Qs, f), (1, 0, 2)).reshape(D_MODEL, D_MODEL)
    ).astype(BFNP)
    kwd = np.ascontiguousarray(
        np.transpose(np.asarray(Ks, f), (1, 0, 2)).reshape(D_MODEL, D_MODEL)
    ).astype(BFNP)
    vwd = np.zeros((D_MODEL, VTOT), f)
    vb = np.zeros((VPAD,), f)
    Vs = np.asarray(Vs, f)
    Vbs = np.asarray(Vbs, f)
    for h in range(N_HEADS):
        vwd[:, h * VW:h * VW + D_HEAD] = Vs[h]
        vb[h * VW:h * VW + D_HEAD] = Vbs[h]
        vb[h * VW + D_HEAD] = 1.0
    vwd = vwd.astype(BFNP)
    owd = np.ascontiguousarray(np.asarray(O, f).reshape(D_MODEL, D_MODEL)).astype(BFNP)
    qbf = np.ascontiguousarray(np.asarray(Qbs, f).reshape(D_MODEL))
    kbf = np.ascontiguousarray(np.asarray(Kbs, f).reshape(D_MODEL))
    obf = np.ascontiguousarray(np.asarray(Ob, f).reshape(D_MODEL))
    return qwd, kwd, vwd, owd, qbf, kbf, vb, obf


def kernel(normalized_resid_pre, encoder_output, Qs, Qbs, Ks, Kbs, Vs, Vbs, O, Ob,
           _trace=False, _trace_kwargs=None):
    nc = _get_program()
    qwd, kwd, vwd, owd, qbf, kbf, vb, obf = _pack_weights(Qs, Qbs, Ks, Kbs, Vs, Vbs, O, Ob)
    enc = np.asarray(encoder_output, np.float32)
    nrp = np.asarray(normalized_resid_pre, np.float32)
    in_maps = []
    for b in range(BATCH):
        in_maps.append({
            "encT": np.ascontiguousarray(enc[b].T).astype(BFNP),
            "nrpT": np.ascontiguousarray(nrp[b].T).astype(BFNP),
            "qwd": qwd, "kwd": kwd, "vwd": vwd, "owd": owd,
            "qb": qbf, "kb": kbf, "vb": vb, "ob": obf,
        })
    res = run_bass_kernel_spmd(
        nc, in_maps, list(range(BATCH)),
        trace=_trace, **(_trace_kwargs or {}),
    )
    out = np.stack([res.results[b]["out"] for b in range(BATCH)], axis=0)
    if _trace:
        _CACHE["last_results"] = res
    return out
